# revision 8
# baseline (speedup 1.0000x reference)
"""Trainium2 Bass kernel for the fuzzy joint-membership layer.

Math (derived from the reference 2-qubit circuit, verified vs oracle):
  out[b, 2p,   c] = 0.5 + 0.5*cos(theta_c)*cos(x0) - 0.5*sin(theta_c)*sin(x0)*sin(x1)
  out[b, 2p+1, c] = 0.5 + 0.5*cos(x0)*cos(x1)
where x0 = xf[b, pair_idx[b,p,0]], x1 = xf[b, pair_idx[b,p,1]].

Sharding: pure data parallel, batch 4096 -> 8 cores x 512 rows.

Gather: gpsimd local_scatter (hardware vector scatter in Q7 local RAM,
per-partition independent indices; ~10 cycles per 32 scattered int16)
instead of ap_gather (~36 cycles/index on the Q7 command interface).
Host precomputes, per row, the FIRST slot wanting each pixel
(idxA[row, pix] = slot or -1) plus lin-lin-wide duplicate-chain maps:
round 0 serves occurrence ordinal 1 (src S0), round 1 ordinal 2 (src
S1), round 2+k ordinals [3*2^k, 3*2^(k+1)) (src = running sum U).
Only one DVE add gates the scatter chain (U2); the rest overlap.

Engine-contention rules honored (DVE 2-port ops block GpSimd):
  - range reduction runs on ACT (Copy with scale/bias implements the
    round-to-nearest magic in turns: n = round(v/2pi), f = v/2pi - n)
  - |f| via ACT Abs; sin/cos via ACT Sin with scale=-2pi
  - DVE runs only tensor_tensor (1-port) + one bf16 stt per tile
Slot layout is half-split (x0 of pair p -> slot p, x1 -> slot 460+p)
so pair reads are unit-stride. Class expansion is c-major against
pre-replicated theta tables. Output tile is fp16; host upcasts (the
output DMA halves). Total error ~5e-3 vs the 2e-2 gate.
"""

import math
import numpy as np

B, PIX, NPAIR, C = 4096, 3072, 460, 10
NG = 2 * NPAIR          # 920 gathered values per row
OUTW = NG * C           # 9200
NCORES = 8
BS = B // NCORES        # 512 rows per core
TILES = BS // 128       # 4

_cache = {}


def _ensure_path():
    try:
        import concourse  # noqa: F401
    except ImportError:
        import sys
        sys.path.insert(0, "/opt/trn_rl_repo")


def build_nc(bs=BS, rounds=3):
    _ensure_path()
    from contextlib import ExitStack
    import concourse.tile as tile
    from concourse import bacc, mybir

    f32, f16, bf16, i16 = (
        mybir.dt.float32, mybir.dt.float16, mybir.dt.bfloat16, mybir.dt.int16
    )
    Sin = mybir.ActivationFunctionType.Sin
    Copy = mybir.ActivationFunctionType.Copy
    Abs = mybir.ActivationFunctionType.Abs
    mult = mybir.AluOpType.mult
    add = mybir.AluOpType.add
    sub_ = mybir.AluOpType.subtract
    maxop = mybir.AluOpType.max
    ntiles = bs // 128
    assert rounds >= 2

    nc = bacc.Bacc("TRN2", target_bir_lowering=False, debug=False)
    x_ext = nc.declare_dram_parameter("x16", [bs, PIX], f16, isOutput=False)
    ia_ext = nc.declare_dram_parameter("ia", [bs, PIX], i16, isOutput=False)
    cc_ext = nc.declare_dram_parameter("cc", [bs, rounds * NG], i16, isOutput=False)
    th_ext = nc.declare_dram_parameter("theta", [128, C], f32, isOutput=False)
    out_ext = nc.declare_dram_parameter("out", [bs, OUTW], f16, isOutput=True)

    PI, TWO_PI = math.pi, 2 * math.pi
    MAGIC, INV2PI = 1.5 * 2 ** 23, 1.0 / (2 * math.pi)

    with tile.TileContext(nc) as tc, ExitStack() as ctx:
        cpool = ctx.enter_context(tc.tile_pool(name="const", bufs=1))
        xpool = ctx.enter_context(tc.tile_pool(name="xf", bufs=2))
        ipool = ctx.enter_context(tc.tile_pool(name="ia", bufs=2))
        kpool = ctx.enter_context(tc.tile_pool(name="cc", bufs=2))
        spool = ctx.enter_context(tc.tile_pool(name="sc", bufs=2))
        upool = ctx.enter_context(tc.tile_pool(name="uc", bufs=2))
        vpool = ctx.enter_context(tc.tile_pool(name="v", bufs=2))
        tpool = ctx.enter_context(tc.tile_pool(name="trig", bufs=2))
        wpool = ctx.enter_context(tc.tile_pool(name="we", bufs=2))
        epool = ctx.enter_context(tc.tile_pool(name="expand", bufs=2))
        opool = ctx.enter_context(tc.tile_pool(name="ot", bufs=2))

        pihalf = cpool.tile([128, 1], f32)
        nc.vector.memset(pihalf[:], PI / 2)

        # theta coefficients: hct = 0.5*cos(theta), nhst = -0.5*sin(theta),
        # replicated c-major into [C, NPAIR] bf16 tables for the expansion.
        th_sb = cpool.tile([128, C], f32)
        nc.sync.dma_start(out=th_sb[:], in_=th_ext[:, :])
        tt1 = cpool.tile([128, C], f32)
        nc.vector.tensor_scalar(tt1[:], th_sb[:], INV2PI, MAGIC, mult, add)
        nc.vector.tensor_scalar(tt1[:], tt1[:], MAGIC, None, sub_)
        tnegr = cpool.tile([128, C], f32)
        nc.vector.scalar_tensor_tensor(tnegr[:], tt1[:], TWO_PI, th_sb[:], mult, sub_)
        nc.vector.tensor_scalar(tt1[:], tnegr[:], -1.0, None, mult)
        nc.vector.tensor_tensor(tt1[:], tt1[:], tnegr[:], maxop)
        cvt = cpool.tile([128, C], f32)
        svNt = cpool.tile([128, C], f32)
        nc.scalar.activation(svNt[:], tnegr[:], Sin, bias=0.0)
        nc.scalar.activation(cvt[:], tt1[:], Sin, bias=pihalf[:, 0:1], scale=-1.0)
        hcoef = cpool.tile([128, 2 * C], f32)
        nc.vector.tensor_scalar(hcoef[:, 0:C], cvt[:], 0.5, None, mult)
        nc.vector.tensor_scalar(hcoef[:, C:2 * C], svNt[:], 0.5, None, mult)
        hrep = cpool.tile([128, C * NPAIR], bf16)
        nrep = cpool.tile([128, C * NPAIR], bf16)
        hrepT = hrep[:].rearrange("p (c a) -> p c a", a=NPAIR)
        nrepT = nrep[:].rearrange("p (c a) -> p c a", a=NPAIR)
        nc.scalar.activation(
            hrepT, hcoef[:, 0:C].unsqueeze(2).broadcast_to([128, C, NPAIR]), Copy
        )
        nc.scalar.activation(
            nrepT, hcoef[:, C:2 * C].unsqueeze(2).broadcast_to([128, C, NPAIR]), Copy
        )

        for t in range(ntiles):
            rows = slice(t * 128, (t + 1) * 128)
            ia = ipool.tile([128, PIX], i16)
            nc.sync.dma_start(out=ia[:], in_=ia_ext[rows, :])
            xf = xpool.tile([128, PIX], f16)
            nc.sync.dma_start(out=xf[:], in_=x_ext[rows, :])
            ct = kpool.tile([128, rounds * NG], i16)
            nc.sync.dma_start(out=ct[:], in_=cc_ext[rows, :])

            # lin-lin-wide scatter rounds; only U2 gates the gpsimd chain
            S = spool.tile([128, (rounds + 1) * NG], f16)
            U = upool.tile([128, (rounds - 1) * NG], f16)
            V = vpool.tile([128, NG], f16)

            def sbuf_S(r):
                return S[:, r * NG:(r + 1) * NG]

            nc.gpsimd.local_scatter(
                sbuf_S(0), xf[:], ia[:],
                channels=128, num_elems=NG, num_idxs=PIX,
            )
            nc.gpsimd.local_scatter(
                sbuf_S(1), sbuf_S(0), ct[:, 0:NG],
                channels=128, num_elems=NG, num_idxs=NG,
            )
            nc.gpsimd.local_scatter(
                sbuf_S(2), sbuf_S(1), ct[:, NG:2 * NG],
                channels=128, num_elems=NG, num_idxs=NG,
            )
            ub = U[:, 0:NG]
            nc.vector.tensor_tensor(ub, sbuf_S(0), sbuf_S(1), add)
            for r in range(2, rounds + 1):
                un = V[:] if r == rounds else U[:, (r - 1) * NG:r * NG]
                nc.vector.tensor_tensor(un, ub, sbuf_S(r), add)
                if r < rounds:
                    nc.gpsimd.local_scatter(
                        sbuf_S(r + 1), un, ct[:, r * NG:(r + 1) * NG],
                        channels=128, num_elems=NG, num_idxs=NG,
                    )
                ub = un

            # range reduction in turns, DVE does only the one subtract:
            # u = v/2pi, n = round(u) via magic, f = u - n in [-.5, .5]
            u = tpool.tile([128, NG], f32, tag="u")
            nc.scalar.activation(u[:], V[:], Copy, scale=INV2PI)
            t1 = tpool.tile([128, NG], f32, tag="t1")
            nc.scalar.activation(t1[:], u[:], Copy, bias=MAGIC)
            nc.scalar.activation(t1[:], t1[:], Copy, bias=-MAGIC)
            f = tpool.tile([128, NG], f16, tag="f")
            nc.vector.tensor_tensor(f[:], u[:], t1[:], sub_)
            fa = tpool.tile([128, NG], f16, tag="fa")
            nc.scalar.activation(fa[:], f[:], Abs)
            # -sin(v) = Sin(-2pi*f); cos(v) = Sin(pi/2 - 2pi*|f|)
            svN = tpool.tile([128, NG], bf16, tag="svN")
            cv = tpool.tile([128, NG], bf16, tag="cv")
            nc.scalar.activation(svN[:], f[:], Sin, scale=-TWO_PI)
            nc.scalar.activation(
                cv[:], fa[:], Sin, bias=pihalf[:, 0:1], scale=-TWO_PI
            )

            # half-split layout: slots [0:460] = x0, [460:920] = x1
            w = wpool.tile([128, NPAIR], bf16, tag="w")
            e = wpool.tile([128, NPAIR], bf16, tag="e")
            nc.vector.tensor_tensor(w[:], svN[:, 0:NPAIR], svN[:, NPAIR:NG], mult)
            nc.vector.tensor_tensor(e[:], cv[:, 0:NPAIR], cv[:, NPAIR:NG], mult)

            # class expansion, c-major: even = (A*hct_c + 0.5) + W*nhst_c
            tev = epool.tile([128, C * NPAIR], bf16, tag="tev")
            tw2 = epool.tile([128, C * NPAIR], bf16, tag="tw2")
            tevT = tev[:].rearrange("p (c a) -> p c a", a=NPAIR)
            tw2T = tw2[:].rearrange("p (c a) -> p c a", a=NPAIR)
            A3 = cv[:, 0:NPAIR].unsqueeze(1).broadcast_to([128, C, NPAIR])
            W3 = w[:].unsqueeze(1).broadcast_to([128, C, NPAIR])
            E3 = e[:].unsqueeze(1).broadcast_to([128, C, NPAIR])
            nc.vector.tensor_tensor(tevT, A3, hrepT, mult)
            nc.vector.tensor_tensor(tw2T, W3, nrepT, mult)

            ot = opool.tile([128, OUTW], f16)
            otv = ot[:].rearrange("p (a b) -> p a b", b=2 * C)
            evT = otv[:, :, 0:C].transpose([0, 2, 1])
            ovT = otv[:, :, C:2 * C].transpose([0, 2, 1])
            nc.vector.scalar_tensor_tensor(evT, tevT, 0.5, tw2T, add, add)
            nc.scalar.activation(ovT, E3, Copy, bias=0.5, scale=0.5)
            nc.sync.dma_start(out=out_ext[rows, :], in_=ot[:])

    nc.compile()
    return nc


def _prep_scatter_maps(pair_idx):
    """Round-0 scatter map + lin-lin-wide duplicate-chain maps.

    Slot layout is half-split: x0 of pair p -> slot p, x1 -> slot 460+p.
    Chain round 0 serves ordinal 1 (src ordinal 0), round 1 serves
    ordinal 2 (src 1), round 2+k serves [3*2^k, 3*2^(k+1)) (src d-3*2^k).
    Returns (idxA [B, PIX] i16, chains [T, B, NG] i16, T>=2).
    """
    pidx = pair_idx.reshape(B, NPAIR, 2)
    idx = np.concatenate([pidx[:, :, 0], pidx[:, :, 1]], axis=1).astype(np.int64)
    j = np.arange(NG, dtype=np.int64)[None, :]
    ordk = np.argsort(idx * 1024 + j, axis=1)      # slots sorted by (pixel, slot)
    px_sorted = np.take_along_axis(idx, ordk, axis=1)
    first = np.ones((B, NG), dtype=bool)
    first[:, 1:] = px_sorted[:, 1:] != px_sorted[:, :-1]
    kk = np.broadcast_to(np.arange(NG, dtype=np.int64), (B, NG))
    run_start = np.maximum.accumulate(np.where(first, kk, 0), axis=1)
    o = kk - run_start                              # occurrence ordinal per sorted pos
    maxmult = int(o.max()) + 1
    T = 2
    while 3 * (1 << (T - 2)) < maxmult:             # capacity(T) = 3*2^(T-2)
        T += 1

    idxA = np.full((B, PIX), -1, np.int16)
    rr, cc = np.nonzero(first)
    idxA[rr, px_sorted[rr, cc]] = ordk[rr, cc]

    chains = np.full((T, B, NG), -1, np.int16)
    rr, cc = np.nonzero(o >= 1)
    d = o[rr, cc]
    t_of = np.zeros(len(d), dtype=np.int64)
    src_off = np.ones(len(d), dtype=np.int64)       # d==1: round 0, src d-1
    m2 = d == 2
    t_of[m2] = 1                                    # d==2: round 1, src ord 1
    for k in range(0, 12):
        lo, hi = 3 << k, 3 << (k + 1)
        mk = (d >= lo) & (d < hi)
        if not mk.any():
            break
        t_of[mk] = 2 + k
        src_off[mk] = 3 << k
    src = ordk[rr, cc - src_off]
    dst = ordk[rr, cc]
    chains[t_of, rr, src] = dst
    return idxA, chains, T


def _get_nc(rounds):
    key = ("nc", rounds)
    if key not in _cache:
        _cache[key] = build_nc(rounds=rounds)
    return _cache[key]


def kernel(x, pair_idx, theta):
    _ensure_path()
    from concourse.bass_utils import run_bass_kernel_spmd

    x16 = np.ascontiguousarray(
        np.asarray(x, dtype=np.float32).reshape(B, PIX).astype(np.float16)
    )
    idxA, chains, T = _prep_scatter_maps(np.asarray(pair_idx))
    nc = _get_nc(T)
    cc = np.ascontiguousarray(
        chains.transpose(1, 0, 2).reshape(B, T * NG)
    )
    thb = np.ascontiguousarray(
        np.tile(np.asarray(theta, dtype=np.float32).reshape(1, C), (128, 1))
    )
    in_maps = [
        {
            "x16": x16[k * BS:(k + 1) * BS],
            "ia": idxA[k * BS:(k + 1) * BS],
            "cc": cc[k * BS:(k + 1) * BS],
            "theta": thb,
        }
        for k in range(NCORES)
    ]
    res = run_bass_kernel_spmd(nc, in_maps, list(range(NCORES))).results
    out = np.concatenate(
        [res[k]["out"].astype(np.float32) for k in range(NCORES)], axis=0
    )
    return out.reshape(B, NG, C)


# revision 10
# speedup vs baseline: 1.2309x; 1.2309x over previous
"""Trainium2 Bass kernel for the fuzzy joint-membership layer.

Math (derived from the reference 2-qubit circuit, verified vs oracle):
  out[b, 2p,   c] = 0.5 + 0.5*cos(theta_c)*cos(x0) - 0.5*sin(theta_c)*sin(x0)*sin(x1)
  out[b, 2p+1, c] = 0.5 + 0.5*cos(x0)*cos(x1)
where x0 = xf[b, pair_idx[b,p,0]], x1 = xf[b, pair_idx[b,p,1]].

Sharding: pure data parallel, batch 4096 -> 8 cores x 512 rows.

Gather: gpsimd local_scatter (hardware vector scatter in Q7 local RAM,
per-partition independent indices; ~10 cycles per 32 scattered int16)
instead of ap_gather (~36 cycles/index on the Q7 command interface).
Host precomputes, per row, the FIRST slot wanting each pixel
(idxA[row, pix] = slot or -1) plus lin-lin-wide duplicate-chain maps:
round 0 serves occurrence ordinal 1 (src S0), round 1 ordinal 2 (src
S1), round 2+k ordinals [3*2^k, 3*2^(k+1)) (src = running sum U).
Only one DVE add gates the scatter chain (U2); the rest overlap.

Engine-contention rules honored (DVE 2-port ops block GpSimd):
  - range reduction runs on ACT (Copy with scale/bias implements the
    round-to-nearest magic in turns: n = round(v/2pi), f = v/2pi - n)
  - |f| via ACT Abs; sin/cos via ACT Sin with scale=-2pi
  - DVE runs only tensor_tensor (1-port) + one bf16 stt per tile
Slot layout is half-split (x0 of pair p -> slot p, x1 -> slot 460+p)
so pair reads are unit-stride. Class expansion is c-major against
pre-replicated theta tables. Output tile is fp16; host upcasts (the
output DMA halves). Total error ~5e-3 vs the 2e-2 gate.
"""

import math
import numpy as np

B, PIX, NPAIR, C = 4096, 3072, 460, 10
NG = 2 * NPAIR          # 920 gathered values per row
OUTW = NG * C           # 9200
NCORES = 8
BS = B // NCORES        # 512 rows per core
TILES = BS // 128       # 4

_cache = {}


def _ensure_path():
    try:
        import concourse  # noqa: F401
    except ImportError:
        import sys
        sys.path.insert(0, "/opt/trn_rl_repo")


def build_nc(bs=BS, rounds=3):
    _ensure_path()
    from contextlib import ExitStack
    import concourse.tile as tile
    from concourse import bacc, mybir

    f32, f16, bf16, i16 = (
        mybir.dt.float32, mybir.dt.float16, mybir.dt.bfloat16, mybir.dt.int16
    )
    Sin = mybir.ActivationFunctionType.Sin
    Copy = mybir.ActivationFunctionType.Copy
    Abs = mybir.ActivationFunctionType.Abs
    mult = mybir.AluOpType.mult
    add = mybir.AluOpType.add
    sub_ = mybir.AluOpType.subtract
    maxop = mybir.AluOpType.max
    ntiles = bs // 128
    assert rounds >= 2

    nc = bacc.Bacc("TRN2", target_bir_lowering=False, debug=False)
    x_ext = nc.declare_dram_parameter("x16", [bs, PIX], f16, isOutput=False)
    ia_ext = nc.declare_dram_parameter("ia", [bs, PIX], i16, isOutput=False)
    cc_ext = nc.declare_dram_parameter("cc", [bs, rounds * NG], i16, isOutput=False)
    th_ext = nc.declare_dram_parameter("theta", [128, C], f32, isOutput=False)
    out_ext = nc.declare_dram_parameter("out", [bs, OUTW], f16, isOutput=True)

    PI, TWO_PI = math.pi, 2 * math.pi
    MAGIC, INV2PI = 1.5 * 2 ** 23, 1.0 / (2 * math.pi)

    with tile.TileContext(nc) as tc, ExitStack() as ctx:
        cpool = ctx.enter_context(tc.tile_pool(name="const", bufs=1))
        xpool = ctx.enter_context(tc.tile_pool(name="xf", bufs=2))
        ipool = ctx.enter_context(tc.tile_pool(name="ia", bufs=2))
        kpool = ctx.enter_context(tc.tile_pool(name="cc", bufs=2))
        spool = ctx.enter_context(tc.tile_pool(name="sc", bufs=2))
        upool = ctx.enter_context(tc.tile_pool(name="uc", bufs=2))
        vpool = ctx.enter_context(tc.tile_pool(name="v", bufs=2))
        tpool = ctx.enter_context(tc.tile_pool(name="trig", bufs=2))
        wpool = ctx.enter_context(tc.tile_pool(name="we", bufs=2))
        epool = ctx.enter_context(tc.tile_pool(name="expand", bufs=2))
        opool = ctx.enter_context(tc.tile_pool(name="ot", bufs=2))

        pihalf = cpool.tile([128, 1], f32)
        nc.vector.memset(pihalf[:], PI / 2)

        # theta coefficients: hct = 0.5*cos(theta), nhst = -0.5*sin(theta),
        # replicated c-major into [C, NPAIR] bf16 tables for the expansion.
        th_sb = cpool.tile([128, C], f32)
        nc.sync.dma_start(out=th_sb[:], in_=th_ext[:, :])
        tt1 = cpool.tile([128, C], f32)
        nc.vector.tensor_scalar(tt1[:], th_sb[:], INV2PI, MAGIC, mult, add)
        nc.vector.tensor_scalar(tt1[:], tt1[:], MAGIC, None, sub_)
        tnegr = cpool.tile([128, C], f32)
        nc.vector.scalar_tensor_tensor(tnegr[:], tt1[:], TWO_PI, th_sb[:], mult, sub_)
        nc.vector.tensor_scalar(tt1[:], tnegr[:], -1.0, None, mult)
        nc.vector.tensor_tensor(tt1[:], tt1[:], tnegr[:], maxop)
        cvt = cpool.tile([128, C], f32)
        svNt = cpool.tile([128, C], f32)
        nc.scalar.activation(svNt[:], tnegr[:], Sin, bias=0.0)
        nc.scalar.activation(cvt[:], tt1[:], Sin, bias=pihalf[:, 0:1], scale=-1.0)
        hcoef = cpool.tile([128, 2 * C], f32)
        nc.vector.tensor_scalar(hcoef[:, 0:C], cvt[:], 0.5, None, mult)
        nc.vector.tensor_scalar(hcoef[:, C:2 * C], svNt[:], 0.5, None, mult)
        hct = hcoef[:, 0:C]        # 0.5*cos(theta)
        nhst = hcoef[:, C:2 * C]   # -0.5*sin(theta) = 0.5*svN

        for t in range(ntiles):
            rows = slice(t * 128, (t + 1) * 128)
            ia = ipool.tile([128, PIX], i16)
            nc.sync.dma_start(out=ia[:], in_=ia_ext[rows, :])
            xf = xpool.tile([128, PIX], f16)
            nc.sync.dma_start(out=xf[:], in_=x_ext[rows, :])
            ct = kpool.tile([128, rounds * NG], i16)
            nc.sync.dma_start(out=ct[:], in_=cc_ext[rows, :])

            # lin-lin-wide scatter rounds; only U2 gates the gpsimd chain
            S = spool.tile([128, (rounds + 1) * NG], f16)
            U = upool.tile([128, (rounds - 1) * NG], f16)
            V = vpool.tile([128, NG], f16)

            def sbuf_S(r):
                return S[:, r * NG:(r + 1) * NG]

            nc.gpsimd.local_scatter(
                sbuf_S(0), xf[:], ia[:],
                channels=128, num_elems=NG, num_idxs=PIX,
            )
            nc.gpsimd.local_scatter(
                sbuf_S(1), sbuf_S(0), ct[:, 0:NG],
                channels=128, num_elems=NG, num_idxs=NG,
            )
            nc.gpsimd.local_scatter(
                sbuf_S(2), sbuf_S(1), ct[:, NG:2 * NG],
                channels=128, num_elems=NG, num_idxs=NG,
            )
            ub = U[:, 0:NG]
            nc.vector.tensor_tensor(ub, sbuf_S(0), sbuf_S(1), add)
            for r in range(2, rounds + 1):
                un = V[:] if r == rounds else U[:, (r - 1) * NG:r * NG]
                nc.vector.tensor_tensor(un, ub, sbuf_S(r), add)
                if r < rounds:
                    nc.gpsimd.local_scatter(
                        sbuf_S(r + 1), un, ct[:, r * NG:(r + 1) * NG],
                        channels=128, num_elems=NG, num_idxs=NG,
                    )
                ub = un

            # range reduction in turns, DVE does only the one subtract:
            # u = v/2pi, n = round(u) via magic, f = u - n in [-.5, .5]
            u = tpool.tile([128, NG], f32, tag="u")
            nc.scalar.activation(u[:], V[:], Copy, scale=INV2PI)
            t1 = tpool.tile([128, NG], f32, tag="t1")
            nc.scalar.activation(t1[:], u[:], Copy, bias=MAGIC)
            nc.scalar.activation(t1[:], t1[:], Copy, bias=-MAGIC)
            f = tpool.tile([128, NG], f16, tag="f")
            nc.vector.tensor_tensor(f[:], u[:], t1[:], sub_)
            fa = tpool.tile([128, NG], f16, tag="fa")
            nc.scalar.activation(fa[:], f[:], Abs)
            # -sin(v) = Sin(-2pi*f); cos(v) = Sin(pi/2 - 2pi*|f|)
            svN = tpool.tile([128, NG], bf16, tag="svN")
            cv = tpool.tile([128, NG], bf16, tag="cv")
            nc.scalar.activation(svN[:], f[:], Sin, scale=-TWO_PI)
            nc.scalar.activation(
                cv[:], fa[:], Sin, bias=pihalf[:, 0:1], scale=-TWO_PI
            )

            # half-split layout: slots [0:460] = x0, [460:920] = x1
            w = wpool.tile([128, NPAIR], bf16, tag="w")
            e = wpool.tile([128, NPAIR], bf16, tag="e")
            nc.vector.tensor_tensor(w[:], svN[:, 0:NPAIR], svN[:, NPAIR:NG], mult)
            nc.vector.tensor_tensor(e[:], cv[:, 0:NPAIR], cv[:, NPAIR:NG], mult)

            # class expansion, pair-major: even = (A*hct_c + 0.5) + W*nhst_c
            tev = epool.tile([128, NPAIR * C], bf16, tag="tev")
            tw2 = epool.tile([128, NPAIR * C], bf16, tag="tw2")
            tev3 = tev[:].rearrange("p (a b) -> p a b", b=C)
            tw23 = tw2[:].rearrange("p (a b) -> p a b", b=C)
            A3 = cv[:, 0:NPAIR].unsqueeze(2).broadcast_to([128, NPAIR, C])
            H3 = hct.unsqueeze(1).broadcast_to([128, NPAIR, C])
            W3 = w[:].unsqueeze(2).broadcast_to([128, NPAIR, C])
            N3 = nhst.unsqueeze(1).broadcast_to([128, NPAIR, C])
            E3 = e[:].unsqueeze(2).broadcast_to([128, NPAIR, C])
            nc.vector.tensor_tensor(tev3, A3, H3, mult)
            nc.vector.tensor_tensor(tw23, W3, N3, mult)

            ot = opool.tile([128, OUTW], f16)
            otv = ot[:].rearrange("p (a b) -> p a b", b=2 * C)
            nc.vector.scalar_tensor_tensor(otv[:, :, 0:C], tev3, 0.5, tw23, add, add)
            nc.scalar.activation(otv[:, :, C:2 * C], E3, Copy, bias=0.5, scale=0.5)
            nc.sync.dma_start(out=out_ext[rows, :], in_=ot[:])

    nc.compile()
    return nc


def _prep_scatter_maps(pair_idx):
    """Round-0 scatter map + lin-lin-wide duplicate-chain maps.

    Slot layout is half-split: x0 of pair p -> slot p, x1 -> slot 460+p.
    Chain round 0 serves ordinal 1 (src ordinal 0), round 1 serves
    ordinal 2 (src 1), round 2+k serves [3*2^k, 3*2^(k+1)) (src d-3*2^k).
    Returns (idxA [B, PIX] i16, chains [T, B, NG] i16, T>=2).
    """
    pidx = pair_idx.reshape(B, NPAIR, 2)
    idx = np.concatenate([pidx[:, :, 0], pidx[:, :, 1]], axis=1).astype(np.int64)
    j = np.arange(NG, dtype=np.int64)[None, :]
    ordk = np.argsort(idx * 1024 + j, axis=1)      # slots sorted by (pixel, slot)
    px_sorted = np.take_along_axis(idx, ordk, axis=1)
    first = np.ones((B, NG), dtype=bool)
    first[:, 1:] = px_sorted[:, 1:] != px_sorted[:, :-1]
    kk = np.broadcast_to(np.arange(NG, dtype=np.int64), (B, NG))
    run_start = np.maximum.accumulate(np.where(first, kk, 0), axis=1)
    o = kk - run_start                              # occurrence ordinal per sorted pos
    maxmult = int(o.max()) + 1
    T = 2
    while 3 * (1 << (T - 2)) < maxmult:             # capacity(T) = 3*2^(T-2)
        T += 1

    idxA = np.full((B, PIX), -1, np.int16)
    rr, cc = np.nonzero(first)
    idxA[rr, px_sorted[rr, cc]] = ordk[rr, cc]

    chains = np.full((T, B, NG), -1, np.int16)
    rr, cc = np.nonzero(o >= 1)
    d = o[rr, cc]
    t_of = np.zeros(len(d), dtype=np.int64)
    src_off = np.ones(len(d), dtype=np.int64)       # d==1: round 0, src d-1
    m2 = d == 2
    t_of[m2] = 1                                    # d==2: round 1, src ord 1
    for k in range(0, 12):
        lo, hi = 3 << k, 3 << (k + 1)
        mk = (d >= lo) & (d < hi)
        if not mk.any():
            break
        t_of[mk] = 2 + k
        src_off[mk] = 3 << k
    src = ordk[rr, cc - src_off]
    dst = ordk[rr, cc]
    chains[t_of, rr, src] = dst
    return idxA, chains, T


def _get_nc(rounds):
    key = ("nc", rounds)
    if key not in _cache:
        _cache[key] = build_nc(rounds=rounds)
    return _cache[key]


def kernel(x, pair_idx, theta):
    _ensure_path()
    from concourse.bass_utils import run_bass_kernel_spmd

    x16 = np.ascontiguousarray(
        np.asarray(x, dtype=np.float32).reshape(B, PIX).astype(np.float16)
    )
    idxA, chains, T = _prep_scatter_maps(np.asarray(pair_idx))
    nc = _get_nc(T)
    cc = np.ascontiguousarray(
        chains.transpose(1, 0, 2).reshape(B, T * NG)
    )
    thb = np.ascontiguousarray(
        np.tile(np.asarray(theta, dtype=np.float32).reshape(1, C), (128, 1))
    )
    in_maps = [
        {
            "x16": x16[k * BS:(k + 1) * BS],
            "ia": idxA[k * BS:(k + 1) * BS],
            "cc": cc[k * BS:(k + 1) * BS],
            "theta": thb,
        }
        for k in range(NCORES)
    ]
    res = run_bass_kernel_spmd(nc, in_maps, list(range(NCORES))).results
    out = np.concatenate(
        [res[k]["out"].astype(np.float32) for k in range(NCORES)], axis=0
    )
    return out.reshape(B, NG, C)


# revision 12
# speedup vs baseline: 1.2321x; 1.0010x over previous
"""Trainium2 Bass kernel for the fuzzy joint-membership layer.

Math (derived from the reference 2-qubit circuit, verified vs oracle):
  out[b, 2p,   c] = 0.5 + 0.5*cos(theta_c)*cos(x0) - 0.5*sin(theta_c)*sin(x0)*sin(x1)
  out[b, 2p+1, c] = 0.5 + 0.5*cos(x0)*cos(x1)
where x0 = xf[b, pair_idx[b,p,0]], x1 = xf[b, pair_idx[b,p,1]].

Sharding: pure data parallel, batch 4096 -> 8 cores x 512 rows.

Gather: gpsimd local_scatter (hardware vector scatter in Q7 local RAM,
per-partition independent indices; ~10 cycles per 32 scattered int16)
instead of ap_gather (~36 cycles/index on the Q7 command interface).
Host precomputes, per row, the FIRST slot wanting each pixel
(idxA[row, pix] = slot or -1) plus lin-lin-wide duplicate-chain maps:
round 0 serves occurrence ordinal 1 (src S0), round 1 ordinal 2 (src
S1), round 2+k ordinals [3*2^k, 3*2^(k+1)) (src = running union U).

Engine-contention rules honored (any 2-SBUF-source DVE op can stall
against a concurrent LocalScatter on the shared read port):
  - x is transported as fp16 of (x + 2pi) -- positive, and cos/sin are
    2pi-periodic so no unbias is ever needed.  Disjoint-support chain
    unions then become MAX tensor_reduce (single-port, never contends,
    fp16 out so it can feed the next scatter round).
  - range reduction in turns on ACT: u = v/2pi, n = round(u) via the
    fp32 magic add; f = u - n computed as an ADD tensor_reduce over
    the adjacent pair [u | -n] (ACT emits -n via scale=-1).
  - |f| via ACT Abs; -sin(v) = Sin(-2pi*f), cos(v) = Sin(pi/2-2pi*|f|)
  - pair products and class expansion are bf16 tensor_tensor ops
Slot layout is half-split (x0 of pair p -> slot p, x1 -> slot 460+p)
so pair reads are unit-stride.  Output tile is fp16, host upcasts.
Total error ~6e-3 vs the 2e-2 gate.
"""

import math
import numpy as np

B, PIX, NPAIR, C = 4096, 3072, 460, 10
NG = 2 * NPAIR          # 920 gathered values per row
OUTW = NG * C           # 9200
NCORES = 8
BS = B // NCORES        # 512 rows per core
TILES = BS // 128       # 4

_cache = {}


def _ensure_path():
    try:
        import concourse  # noqa: F401
    except ImportError:
        import sys
        sys.path.insert(0, "/opt/trn_rl_repo")


def build_nc(bs=BS, rounds=3):
    _ensure_path()
    from contextlib import ExitStack
    import concourse.tile as tile
    from concourse import bacc, mybir

    f32, f16, bf16, i16 = (
        mybir.dt.float32, mybir.dt.float16, mybir.dt.bfloat16, mybir.dt.int16
    )
    Sin = mybir.ActivationFunctionType.Sin
    Copy = mybir.ActivationFunctionType.Copy
    Abs = mybir.ActivationFunctionType.Abs
    mult = mybir.AluOpType.mult
    add = mybir.AluOpType.add
    sub_ = mybir.AluOpType.subtract
    maxop = mybir.AluOpType.max
    X = mybir.AxisListType.X
    ntiles = bs // 128
    assert rounds >= 2

    nc = bacc.Bacc("TRN2", target_bir_lowering=False, debug=False)
    x_ext = nc.declare_dram_parameter("x16", [bs, PIX], f16, isOutput=False)
    ia_ext = nc.declare_dram_parameter("ia", [bs, PIX], i16, isOutput=False)
    cc_ext = nc.declare_dram_parameter("cc", [bs, rounds * NG], i16, isOutput=False)
    th_ext = nc.declare_dram_parameter("theta", [128, C], f32, isOutput=False)
    out_ext = nc.declare_dram_parameter("out", [bs, OUTW], f16, isOutput=True)

    PI, TWO_PI = math.pi, 2 * math.pi
    MAGIC, INV2PI = 1.5 * 2 ** 23, 1.0 / (2 * math.pi)

    # chain buffer slot order: S0, S1, U1, S2, U2, S3 (unions next to
    # their reduce sources so MAX tensor_reduce reads adjacent slices)
    NCH = 2 * rounds

    with tile.TileContext(nc) as tc, ExitStack() as ctx:
        cpool = ctx.enter_context(tc.tile_pool(name="const", bufs=1))
        xpool = ctx.enter_context(tc.tile_pool(name="xf", bufs=2))
        ipool = ctx.enter_context(tc.tile_pool(name="ia", bufs=2))
        kpool = ctx.enter_context(tc.tile_pool(name="cc", bufs=2))
        spool = ctx.enter_context(tc.tile_pool(name="sc", bufs=2))
        tpool = ctx.enter_context(tc.tile_pool(name="trig", bufs=2))
        wpool = ctx.enter_context(tc.tile_pool(name="we", bufs=2))
        epool = ctx.enter_context(tc.tile_pool(name="expand", bufs=2))
        opool = ctx.enter_context(tc.tile_pool(name="ot", bufs=2))

        pihalf = cpool.tile([128, 1], f32)
        nc.vector.memset(pihalf[:], PI / 2)

        # theta coefficients: hct = 0.5*cos(theta), nhst = -0.5*sin(theta)
        th_sb = cpool.tile([128, C], f32)
        nc.sync.dma_start(out=th_sb[:], in_=th_ext[:, :])
        tt1 = cpool.tile([128, C], f32)
        nc.vector.tensor_scalar(tt1[:], th_sb[:], INV2PI, MAGIC, mult, add)
        nc.vector.tensor_scalar(tt1[:], tt1[:], MAGIC, None, sub_)
        tnegr = cpool.tile([128, C], f32)
        nc.vector.scalar_tensor_tensor(tnegr[:], tt1[:], TWO_PI, th_sb[:], mult, sub_)
        nc.vector.tensor_scalar(tt1[:], tnegr[:], -1.0, None, mult)
        nc.vector.tensor_tensor(tt1[:], tt1[:], tnegr[:], maxop)
        cvt = cpool.tile([128, C], f32)
        svNt = cpool.tile([128, C], f32)
        nc.scalar.activation(svNt[:], tnegr[:], Sin, bias=0.0)
        nc.scalar.activation(cvt[:], tt1[:], Sin, bias=pihalf[:, 0:1], scale=-1.0)
        hcoef = cpool.tile([128, 2 * C], f32)
        nc.vector.tensor_scalar(hcoef[:, 0:C], cvt[:], 0.5, None, mult)
        nc.vector.tensor_scalar(hcoef[:, C:2 * C], svNt[:], 0.5, None, mult)
        hct = hcoef[:, 0:C]        # 0.5*cos(theta)
        nhst = hcoef[:, C:2 * C]   # -0.5*sin(theta) = 0.5*svN

        for t in range(ntiles):
            rows = slice(t * 128, (t + 1) * 128)
            ia = ipool.tile([128, PIX], i16)
            nc.sync.dma_start(out=ia[:], in_=ia_ext[rows, :])
            xf = xpool.tile([128, PIX], f16)
            nc.sync.dma_start(out=xf[:], in_=x_ext[rows, :])
            ct = kpool.tile([128, rounds * NG], i16)
            nc.sync.dma_start(out=ct[:], in_=cc_ext[rows, :])

            # scatter chain with MAX-reduce unions (all values > 0)
            ch = spool.tile([128, NCH * NG], f16)

            def sl(i, n=1):
                return ch[:, i * NG:(i + n) * NG]

            def red2(dst, i):  # dst = max over chain slots [i, i+1]
                nc.vector.tensor_reduce(
                    dst, sl(i, 2).rearrange("p (r a) -> p a r", r=2), X, maxop
                )

            nc.gpsimd.local_scatter(
                sl(0), xf[:], ia[:], channels=128, num_elems=NG, num_idxs=PIX,
            )
            nc.gpsimd.local_scatter(
                sl(1), sl(0), ct[:, 0:NG],
                channels=128, num_elems=NG, num_idxs=NG,
            )
            # slot layout: 0:S0 1:S1 | 2:U1 3:S2 | 4:U2 5:S3 | ...
            for r in range(2, rounds + 1):
                red2(sl(2 * r - 2), 2 * r - 4)          # U_{r-1}
                nc.gpsimd.local_scatter(
                    sl(2 * r - 1), sl(2 * r - 3) if r == 2 else sl(2 * r - 2),
                    ct[:, (r - 1) * NG:r * NG],
                    channels=128, num_elems=NG, num_idxs=NG,
                )
            V = wpool.tile([128, NG], f16, tag="V")
            red2(V[:], NCH - 2)

            # range reduction in turns: u = v/2pi, n = round(u), f = u - n
            un = tpool.tile([128, 2 * NG], f32, tag="un")
            u, nneg = un[:, 0:NG], un[:, NG:2 * NG]
            nc.scalar.activation(u, V[:], Copy, scale=INV2PI)
            nc.scalar.activation(nneg, u, Copy, bias=MAGIC)
            nc.scalar.activation(nneg, nneg, Copy, bias=MAGIC, scale=-1.0)  # -n
            f = tpool.tile([128, NG], f32, tag="f")
            nc.vector.tensor_reduce(
                f[:], un[:].rearrange("p (r a) -> p a r", r=2), X, add
            )
            fa = tpool.tile([128, NG], f16, tag="fa")
            nc.scalar.activation(fa[:], f[:], Abs)
            # -sin(v) = Sin(-2pi*f); cos(v) = Sin(pi/2 - 2pi*|f|)
            svN = tpool.tile([128, NG], bf16, tag="svN")
            cv = tpool.tile([128, NG], bf16, tag="cv")
            nc.scalar.activation(svN[:], f[:], Sin, scale=-TWO_PI)
            nc.scalar.activation(
                cv[:], fa[:], Sin, bias=pihalf[:, 0:1], scale=-TWO_PI
            )

            # half-split layout: slots [0:460] = x0, [460:920] = x1
            w = wpool.tile([128, NPAIR], bf16, tag="w")
            e = wpool.tile([128, NPAIR], bf16, tag="e")
            nc.vector.tensor_tensor(w[:], svN[:, 0:NPAIR], svN[:, NPAIR:NG], mult)
            nc.vector.tensor_tensor(e[:], cv[:, 0:NPAIR], cv[:, NPAIR:NG], mult)

            # class expansion, pair-major: even = (A*hct_c + 0.5) + W*nhst_c
            tev = epool.tile([128, NPAIR * C], bf16, tag="tev")
            tw2 = epool.tile([128, NPAIR * C], bf16, tag="tw2")
            tev3 = tev[:].rearrange("p (a b) -> p a b", b=C)
            tw23 = tw2[:].rearrange("p (a b) -> p a b", b=C)
            A3 = cv[:, 0:NPAIR].unsqueeze(2).broadcast_to([128, NPAIR, C])
            H3 = hct.unsqueeze(1).broadcast_to([128, NPAIR, C])
            W3 = w[:].unsqueeze(2).broadcast_to([128, NPAIR, C])
            N3 = nhst.unsqueeze(1).broadcast_to([128, NPAIR, C])
            E3 = e[:].unsqueeze(2).broadcast_to([128, NPAIR, C])
            nc.vector.tensor_tensor(tev3, A3, H3, mult)
            nc.vector.tensor_tensor(tw23, W3, N3, mult)

            ot = opool.tile([128, OUTW], f16)
            otv = ot[:].rearrange("p (a b) -> p a b", b=2 * C)
            nc.vector.scalar_tensor_tensor(otv[:, :, 0:C], tev3, 0.5, tw23, add, add)
            nc.scalar.activation(otv[:, :, C:2 * C], E3, Copy, bias=0.5, scale=0.5)
            nc.sync.dma_start(out=out_ext[rows, :], in_=ot[:])

    nc.compile()
    return nc


def _prep_scatter_maps(pair_idx):
    """Round-0 scatter map + lin-lin-wide duplicate-chain maps.

    Slot layout is half-split: x0 of pair p -> slot p, x1 -> slot 460+p.
    Chain round 0 serves ordinal 1 (src ordinal 0), round 1 serves
    ordinal 2 (src 1), round 2+k serves [3*2^k, 3*2^(k+1)) (src d-3*2^k).
    Returns (idxA [B, PIX] i16, chains [T, B, NG] i16, T>=2).
    """
    pidx = pair_idx.reshape(B, NPAIR, 2)
    idx = np.concatenate([pidx[:, :, 0], pidx[:, :, 1]], axis=1).astype(np.int64)
    j = np.arange(NG, dtype=np.int64)[None, :]
    ordk = np.argsort(idx * 1024 + j, axis=1)      # slots sorted by (pixel, slot)
    px_sorted = np.take_along_axis(idx, ordk, axis=1)
    first = np.ones((B, NG), dtype=bool)
    first[:, 1:] = px_sorted[:, 1:] != px_sorted[:, :-1]
    kk = np.broadcast_to(np.arange(NG, dtype=np.int64), (B, NG))
    run_start = np.maximum.accumulate(np.where(first, kk, 0), axis=1)
    o = kk - run_start                              # occurrence ordinal per sorted pos
    maxmult = int(o.max()) + 1
    T = 2
    while 3 * (1 << (T - 2)) < maxmult:             # capacity(T) = 3*2^(T-2)
        T += 1

    idxA = np.full((B, PIX), -1, np.int16)
    rr, cc = np.nonzero(first)
    idxA[rr, px_sorted[rr, cc]] = ordk[rr, cc]

    chains = np.full((T, B, NG), -1, np.int16)
    rr, cc = np.nonzero(o >= 1)
    d = o[rr, cc]
    t_of = np.zeros(len(d), dtype=np.int64)
    src_off = np.ones(len(d), dtype=np.int64)       # d==1: round 0, src d-1
    t_of[d == 2] = 1                                # d==2: round 1, src ord 1
    for k in range(0, 12):
        mk = (d >= (3 << k)) & (d < (3 << (k + 1)))
        if not mk.any():
            break
        t_of[mk] = 2 + k
        src_off[mk] = 3 << k
    src = ordk[rr, cc - src_off]
    dst = ordk[rr, cc]
    chains[t_of, rr, src] = dst
    return idxA, chains, T


def _get_nc(rounds):
    key = ("nc", rounds)
    if key not in _cache:
        _cache[key] = build_nc(rounds=rounds)
    return _cache[key]


def kernel(x, pair_idx, theta):
    _ensure_path()
    from concourse.bass_utils import run_bass_kernel_spmd

    # transport x as fp16 of (x + 2pi): positive for MAX-reduce unions,
    # and exactly absorbed by the 2pi-periodic trig downstream
    x16 = np.ascontiguousarray(
        (np.asarray(x, dtype=np.float32).reshape(B, PIX) + np.float32(2 * math.pi))
        .astype(np.float16)
    )
    idxA, chains, T = _prep_scatter_maps(np.asarray(pair_idx))
    nc = _get_nc(T)
    cc = np.ascontiguousarray(
        chains.transpose(1, 0, 2).reshape(B, T * NG)
    )
    thb = np.ascontiguousarray(
        np.tile(np.asarray(theta, dtype=np.float32).reshape(1, C), (128, 1))
    )
    in_maps = [
        {
            "x16": x16[k * BS:(k + 1) * BS],
            "ia": idxA[k * BS:(k + 1) * BS],
            "cc": cc[k * BS:(k + 1) * BS],
            "theta": thb,
        }
        for k in range(NCORES)
    ]
    res = run_bass_kernel_spmd(nc, in_maps, list(range(NCORES))).results
    out = np.concatenate(
        [res[k]["out"].astype(np.float32) for k in range(NCORES)], axis=0
    )
    return out.reshape(B, NG, C)
